# revision 2
# baseline (speedup 1.0000x reference)
"""DTW kernel for nn_DTW_56272661512310.

Sharding (per hint): data-parallel over batch B=64 across 8 NeuronCores
(8 samples per core); scalars a, b replicated.  Inputs are sharded on
host, each core computes tanh(a*cos_sim+b) for its samples and the DTW
DP, output gathered to [64].

Math reformulation of the reference DP (avoids cummax in the loop):
  reference:  P[:,0]=sim[:,0]; P[0,j]=relu(sim[0,j]);
              P[i,j] = max(P[0:i, j-1]) + relu(sim[i,j])
  With M[i,j] = cummax_i(P[:, j]):
              M[i,j] = max(M[i-1,j], M[i-1,j-1] + relu(sim[i,j]))
              M[i,0] = max(M[i-1,0], sim[i,0])
  -> row-scan over i, uniform update M_new = max(M, H + R') where
     H = [0, M[:-1]] and R' is the row with col 0 raw, cols>=1 relu'd.
  Answer = (M[I-2, J-2] + relu(sim[I-1, J-1])) / J.
"""

import numpy as np
import jax
import jax.numpy as jnp
from functools import partial

B, I, J, D = 64, 512, 384, 768
EPS = 1e-8
N_CORES = 8
B_PER = B // N_CORES


@partial(jax.pmap, axis_name="x")
def _sim_dev(e1, e2, a, b):
    # e1: [b, I, D], e2: [b, J, D] -> tanh(cos_sim * a + b): [b, I, J]
    n1 = e1 / jnp.clip(jnp.linalg.norm(e1, axis=-1, keepdims=True), EPS)
    n2 = e2 / jnp.clip(jnp.linalg.norm(e2, axis=-1, keepdims=True), EPS)
    sim = jnp.einsum("bid,bjd->bij", n1, n2)
    return jnp.tanh(sim * a[0] + b[0])


@partial(jax.pmap, axis_name="x")
def _dtw_dev(sim):
    # sim: [b, I, J] -> [b]
    R = jnp.maximum(sim, 0.0)
    Rp = jnp.concatenate([sim[:, :, :1], R[:, :, 1:]], axis=2)
    M0 = Rp[:, 0, :]

    def step(M, rp_row):
        H = jnp.pad(M[:, :-1], ((0, 0), (1, 0)))
        return jnp.maximum(M, H + rp_row), None

    xs = jnp.moveaxis(Rp[:, 1 : I - 1, :], 1, 0)  # [I-2, b, J]
    Mf, _ = jax.lax.scan(step, M0, xs)
    return (Mf[:, J - 2] + R[:, I - 1, J - 1]) / J


@partial(jax.pmap, axis_name="x")
def _fused_dev(e1, e2, a, b):
    n1 = e1 / jnp.clip(jnp.linalg.norm(e1, axis=-1, keepdims=True), EPS)
    n2 = e2 / jnp.clip(jnp.linalg.norm(e2, axis=-1, keepdims=True), EPS)
    sim = jnp.einsum("bid,bjd->bij", n1, n2)
    sim = jnp.tanh(sim * a[0] + b[0])
    R = jnp.maximum(sim, 0.0)
    Rp = jnp.concatenate([sim[:, :, :1], R[:, :, 1:]], axis=2)
    M0 = Rp[:, 0, :]

    def step(M, rp_row):
        H = jnp.pad(M[:, :-1], ((0, 0), (1, 0)))
        return jnp.maximum(M, H + rp_row), None

    xs = jnp.moveaxis(Rp[:, 1 : I - 1, :], 1, 0)
    Mf, _ = jax.lax.scan(step, M0, xs)
    return (Mf[:, J - 2] + R[:, I - 1, J - 1]) / J


def _dtw_np(sim):
    # sim: [B, I, J] float32 -> [B]
    sim = np.asarray(sim, dtype=np.float32)
    R = np.maximum(sim, 0.0)
    Rp = R.copy()
    Rp[:, :, 0] = sim[:, :, 0]
    M = Rp[:, 0, :].copy()
    H = np.empty_like(M)
    for i in range(1, I - 1):
        H[:, 1:] = M[:, :-1]
        H[:, 0] = 0.0
        np.maximum(M, H + Rp[:, i, :], out=M)
    return (M[:, J - 2] + R[:, I - 1, J - 1]) / np.float32(J)


_mode = {"dtw": None}  # "dev", "np"


def kernel(emb1, emb2, a, b):
    e1 = np.asarray(emb1, dtype=np.float32).reshape(N_CORES, B_PER, I, D)
    e2 = np.asarray(emb2, dtype=np.float32).reshape(N_CORES, B_PER, J, D)
    aa = np.broadcast_to(np.asarray(a, dtype=np.float32), (N_CORES, 1))
    bb = np.broadcast_to(np.asarray(b, dtype=np.float32), (N_CORES, 1))

    sim = _sim_dev(e1, e2, aa, bb)  # [8, b, I, J] on device

    if _mode["dtw"] in (None, "dev"):
        try:
            out = np.asarray(_dtw_dev(sim))
            _mode["dtw"] = "dev"
            return out.reshape(B).astype(np.float32)
        except Exception:
            _mode["dtw"] = "np"

    sim_h = np.asarray(sim).reshape(B, I, J)
    return _dtw_np(sim_h).reshape(B).astype(np.float32)


if __name__ == "__main__":
    rng = np.random.default_rng(0)
    inputs = dict(
        emb1=rng.standard_normal((B, I, D), dtype=np.float32),
        emb2=rng.standard_normal((B, J, D), dtype=np.float32),
        a=rng.random((1,), dtype=np.float32),
        b=rng.random((1,), dtype=np.float32),
    )
    out = kernel(**inputs)
    print("mode:", _mode, "out[:4]:", out[:4])


# revision 4
# speedup vs baseline: 13.3706x; 13.3706x over previous
"""DTW kernel for nn_DTW_56272661512310.

Sharding (per hint): data-parallel over batch B=64 across 8 NeuronCores
(8 samples per core); scalars a, b replicated.  Inputs are sharded on
host, each core computes tanh(a*cos_sim+b) for its samples and the DTW
DP, output gathered to [64].

Math reformulation of the reference DP (avoids cummax in the loop):
  reference:  P[:,0]=sim[:,0]; P[0,j]=relu(sim[0,j]);
              P[i,j] = max(P[0:i, j-1]) + relu(sim[i,j])
  With M[i,j] = cummax_i(P[:, j]):
              M[i,j] = max(M[i-1,j], M[i-1,j-1] + relu(sim[i,j]))
              M[i,0] = max(M[i-1,0], sim[i,0])
  -> row-scan over i, uniform update M_new = max(M, H + R') where
     H = [0, M[:-1]] and R' is the row with col 0 raw, cols>=1 relu'd.
  Answer = (M[I-2, J-2] + relu(sim[I-1, J-1])) / J.
"""

import numpy as np
import jax
import jax.numpy as jnp
from functools import partial

B, I, J, D = 64, 512, 384, 768
EPS = 1e-8
N_CORES = 8
B_PER = B // N_CORES


@partial(jax.pmap, axis_name="x")
def _sim_dev(e1, e2, a, b):
    # e1: [b, I, D], e2: [b, J, D] -> tanh(cos_sim * a + b): [I, b, J]
    n1 = e1 / jnp.clip(jnp.linalg.norm(e1, axis=-1, keepdims=True), EPS)
    n2 = e2 / jnp.clip(jnp.linalg.norm(e2, axis=-1, keepdims=True), EPS)
    sim = jnp.einsum("bid,bjd->ibj", n1, n2)
    return jnp.tanh(sim * a[0] + b[0])


@partial(jax.pmap, axis_name="x")
def _dtw_dev(sim):
    # sim: [I, b, J] -> [b]
    R = jnp.maximum(sim, 0.0)
    Rp = jnp.concatenate([sim[:, :, :1], R[:, :, 1:]], axis=2)
    M0 = Rp[0]  # [b, J]

    def step(M, rp_row):
        H = jnp.pad(M[:, :-1], ((0, 0), (1, 0)))
        return jnp.maximum(M, H + rp_row), None

    xs = Rp[1 : I - 1]  # [I-2, b, J] — already scan-ordered
    Mf, _ = jax.lax.scan(step, M0, xs, unroll=16)
    return (Mf[:, J - 2] + R[I - 1, :, J - 1]) / J


@partial(jax.pmap, axis_name="x")
def _fused_dev(e1, e2, a, b):
    n1 = e1 / jnp.clip(jnp.linalg.norm(e1, axis=-1, keepdims=True), EPS)
    n2 = e2 / jnp.clip(jnp.linalg.norm(e2, axis=-1, keepdims=True), EPS)
    sim = jnp.einsum("bid,bjd->bij", n1, n2)
    sim = jnp.tanh(sim * a[0] + b[0])
    R = jnp.maximum(sim, 0.0)
    Rp = jnp.concatenate([sim[:, :, :1], R[:, :, 1:]], axis=2)
    M0 = Rp[:, 0, :]

    def step(M, rp_row):
        H = jnp.pad(M[:, :-1], ((0, 0), (1, 0)))
        return jnp.maximum(M, H + rp_row), None

    xs = jnp.moveaxis(Rp[:, 1 : I - 1, :], 1, 0)
    Mf, _ = jax.lax.scan(step, M0, xs)
    return (Mf[:, J - 2] + R[:, I - 1, J - 1]) / J


def _dtw_np(sim):
    # sim: [B, I, J] float32 -> [B]
    sim = np.asarray(sim, dtype=np.float32)
    R = np.maximum(sim, 0.0)
    Rp = R.copy()
    Rp[:, :, 0] = sim[:, :, 0]
    M = Rp[:, 0, :].copy()
    H = np.empty_like(M)
    for i in range(1, I - 1):
        H[:, 1:] = M[:, :-1]
        H[:, 0] = 0.0
        np.maximum(M, H + Rp[:, i, :], out=M)
    return (M[:, J - 2] + R[:, I - 1, J - 1]) / np.float32(J)


_mode = {"dtw": None}  # "dev", "np"


def kernel(emb1, emb2, a, b):
    e1 = np.asarray(emb1, dtype=np.float32).reshape(N_CORES, B_PER, I, D)
    e2 = np.asarray(emb2, dtype=np.float32).reshape(N_CORES, B_PER, J, D)
    aa = np.broadcast_to(np.asarray(a, dtype=np.float32), (N_CORES, 1))
    bb = np.broadcast_to(np.asarray(b, dtype=np.float32), (N_CORES, 1))

    sim = _sim_dev(e1, e2, aa, bb)  # [8, b, I, J] on device

    if _mode["dtw"] in (None, "dev"):
        try:
            out = np.asarray(_dtw_dev(sim))
            _mode["dtw"] = "dev"
            return out.reshape(B).astype(np.float32)
        except Exception:
            _mode["dtw"] = "np"

    sim_h = np.asarray(sim).transpose(0, 2, 1, 3).reshape(B, I, J)
    return _dtw_np(sim_h).reshape(B).astype(np.float32)


if __name__ == "__main__":
    rng = np.random.default_rng(0)
    inputs = dict(
        emb1=rng.standard_normal((B, I, D), dtype=np.float32),
        emb2=rng.standard_normal((B, J, D), dtype=np.float32),
        a=rng.random((1,), dtype=np.float32),
        b=rng.random((1,), dtype=np.float32),
    )
    out = kernel(**inputs)
    print("mode:", _mode, "out[:4]:", out[:4])
